# revision 91
# baseline (speedup 1.0000x reference)
"""LocallyHierarchicalNet Trainium2 kernel.

Net: 10 locally-connected conv1d layers (kernel=stride=2, unshared weights
per position), B=128, C_in=3, H=256, D=1024, then mean + linear head.

Strategy (8 NeuronCores, SPMD):
  - Position-shard layers 0-6: core i owns output positions [i*64,(i+1)*64)
    of layer 0, which narrows to exactly 1 position at layer 6 with zero
    cross-core traffic (binary-tree locality).
  - AllGather the 8 layer-6 outputs (fp16, 64KB each) via shared DRAM, then
    every core redundantly computes layers 7-9 + head (tiny; weights
    prefetched before the collective).
  - All weights and activations are fp16 (PSUM accumulates fp32), halving
    HBM traffic vs fp32 -- the kernel is HBM-bound on streaming the
    position-unshared weights (~17.7 MB/core fp16).
  - Matmul layout: weights are the stationary operand (lhsT [cc=128, o=128]
    chunks, host-pretransposed), activations stream as rhs [cc, B]; each
    position's output lands in PSUM already in chain layout [o, B], so no
    PE transposes anywhere.  ReLU+scale+fp16-cast runs on ScalarE/VectorE
    straight from PSUM into the next layer's SBUF tile -- batched 4
    positions per op on the wide layers (throughput), split across both
    engines on the narrow late layers (latency).
  - PE warmup management: the tensor engine downclocks (2.4 -> 1.2/0.65
    GHz) after idling a few us; cheap no-consumer filler matmuls bridge
    the AllGather wait so the latency-critical tail layers run at full
    clock.
"""

import sys

sys.path.insert(0, "/opt/trn_rl_repo")

import numpy as np

N_CORES = 8
B = 128
C_IN = 3
H = 256
OUT = 10

# per-core output positions per layer (layers 1..9)
NPOS = {1: 32, 2: 16, 3: 8, 4: 4, 5: 2, 6: 1, 7: 4, 8: 2, 9: 1}
# weight slab size (positions per DMA) per layer (layers 1..6); the late
# small layers use fine slabs so their first positions' matmuls overlap the
# remainder of the transfer (they gate the pre-collective latency chain)
SLAB = {1: 8, 2: 8, 3: 4, 4: 2, 5: 1, 6: 1}
# PE filler matmuls (213ns each warm) bridging the AllGather wait so the
# tensor engine does not downclock (TRN2 HAM throttles after ~2.5us idle)
# before the latency-critical tail layers run
FILL_CC = 36
# fp16 weight elems per position per partition: oh(2)*kk(2)*ch(2)*o(128)
WPP = 1024

_NC = {}


def _build(sim=False):
    import concourse.bacc as bacc
    import concourse.mybir as mybir
    import concourse.tile as tile

    dt = mybir.dt
    f32 = dt.float32
    f16 = dt.float16
    Relu = mybir.ActivationFunctionType.Relu

    nc = bacc.Bacc(
        "TRN2",
        target_bir_lowering=False,
        debug=False,
        num_devices=1 if sim else N_CORES,
    )

    xw0_d = nc.dram_tensor("xw0", [6, 64 * (B + H)], f16, kind="ExternalInput")
    w_d = {}
    for l in range(1, 7):
        w_d[l] = nc.dram_tensor(
            f"w{l}", [128, NPOS[l] * WPP], f16, kind="ExternalInput"
        )
    wtail_d = nc.dram_tensor("wtail", [128, 7 * WPP], f16, kind="ExternalInput")
    beta_d = nc.dram_tensor("beta", [128, 2 * OUT], f16, kind="ExternalInput")
    out_d = nc.dram_tensor("out", [B, OUT], f32, kind="ExternalOutput")

    with tile.TileContext(nc) as tc:
        with (
            tc.tile_pool(name="sb", bufs=1) as sb,
            tc.tile_pool(name="wp", bufs=5) as wp,
            tc.tile_pool(name="psp", bufs=2, space="PSUM") as psp,
            tc.tile_pool(name="pss", bufs=3, space="PSUM") as pss,
            tc.tile_pool(name="psf", bufs=1, space="PSUM") as psf,
            tc.tile_pool(name="yp", bufs=2) as yp,
            tc.tile_pool(name="dram", bufs=1, space="DRAM") as dp,
        ):
            # ---- small input loads (sync queue, ahead of weight slabs)
            # flat [p, N] tiles keep the DMA innermost run = whole partition
            # row (>=512B avoids the small-descriptor DMA penalty)
            xw0 = sb.tile([6, 64 * (B + H)], f16, tag="xw0", name="xw0_sb")
            nc.sync.dma_start(xw0[:], xw0_d[:])
            W0OFF = 64 * B  # w0 columns start here inside xw0
            beta_sb = sb.tile([128, 2 * OUT], f16, tag="beta", name="beta_sb")
            nc.scalar.dma_start(beta_sb[:], beta_d[:])

            fillps = psf.tile([128, 512], f32, tag="fill", name="fillps")

            def pe_fill(k, dep, rhs):
                """k no-consumer matmuls (213ns each warm) that keep the PE
                clocked up across a stall. dep (an SBUF [128, B] AP) is read
                as lhsT so the scheduler cannot hoist them earlier than its
                producer; rhs is any [128, 512] resident fp16 AP."""
                for _ in range(k):
                    nc.tensor.matmul(
                        fillps[:], dep, rhs,
                        start=True, stop=True,
                    )

            def relu_group(Xout, g, bs, pt, scale):
                """ReLU(scale*psum[bs positions]) -> fp16 chain layout,
                alternating ScalarE/VectorE per group."""
                dst = Xout[:, g * bs : (g + 1) * bs, :, :]
                if g % 2 == 0:
                    nc.scalar.activation(dst, pt[:], Relu, scale=scale)
                else:
                    nc.vector.tensor_scalar(
                        dst, pt[:], scale, 0.0,
                        mybir.AluOpType.mult, mybir.AluOpType.max,
                    )

            # ---- layer 0: contraction K=6 (=C_in*FS); lhsT = w0 [6, o-half],
            # rhs = x0 [6, B]; out lands as [o, B] chain layout directly.
            X1 = sb.tile([128, 64, 2, B], f16, tag="x1", name="X1")
            s3 = 1.0 / (3.0**0.5)
            for g in range(16):
                pt = psp.tile([128, 4, 2, B], f32, tag="pt", name=f"p0_{g}")
                for i in range(4):
                    pos = g * 4 + i
                    w0c = W0OFF + pos * H
                    for oh in range(2):
                        nc.tensor.matmul(
                            pt[:, i, oh, :],
                            xw0[:, w0c + oh * 128 : w0c + (oh + 1) * 128],
                            xw0[:, pos * B : (pos + 1) * B],
                            start=True,
                            stop=True,
                        )
                relu_group(X1, g, 4, pt, s3)

            s16 = 1.0 / 16.0

            def mm_pos(pt, i, ws, woff, xin, p):
                """One position's 8 accumulating matmuls into pt[:, i]:
                chunk (oh, kk, ch) at flat offset woff + (oh*4+kk*2+ch)*128.
                xin is either a unified tile ([128, pos, ch, B], indexed
                2p+kk) or an (even, odd) tile pair (indexed p)."""
                for oh in range(2):
                    for kk in range(2):
                        if isinstance(xin, tuple):
                            src = xin[kk][:, p, :, :]
                        else:
                            src = xin[:, 2 * p + kk, :, :]
                        for ch in range(2):
                            c0 = woff + (oh * 4 + kk * 2 + ch) * 128
                            nc.tensor.matmul(
                                pt[:, i, oh, :],
                                ws[:, c0 : c0 + 128],
                                src[:, ch, :],
                                start=(kk == 0 and ch == 0),
                                stop=(kk == 1 and ch == 1),
                            )

            def lc_wide(l, Xin, Xout, ws, p0, n):
                """Throughput path (layers 1-2): groups of 4 positions, one
                batched ReLU per group alternating whole-group engines."""
                for g0 in range(0, n, 4):
                    pt = psp.tile([128, 4, 2, B], f32, tag="pt",
                                  name=f"pt{l}_{p0 + g0}")
                    for i in range(4):
                        mm_pos(pt, i, ws, (g0 + i) * WPP, Xin, p0 + g0 + i)
                    dst = Xout[:, p0 + g0 : p0 + g0 + 4, :, :]
                    if (p0 + g0) % 8 < 4:
                        nc.scalar.activation(dst, pt[:], Relu, scale=s16)
                    else:
                        nc.vector.tensor_scalar(
                            dst, pt[:], s16, 0.0,
                            mybir.AluOpType.mult, mybir.AluOpType.max,
                        )

            def split_tiles(l, n, xtag):
                """(even, odd) output tiles for the latency path -- exactly
                the next layer's (kk=0, kk=1) operands."""
                ev = sb.tile([128, (n + 1) // 2, 2, B], f16,
                             tag=f"{xtag}e", name=f"X{l + 1}e")
                od = None
                if n > 1:
                    od = sb.tile([128, n // 2, 2, B], f16,
                                 tag=f"{xtag}o", name=f"X{l + 1}o")
                return ev, od

            def lc_split(l, xin, ws, woff0, p0, nslab, ev, od):
                """Latency path (layers 3-9): one PSUM tile and one ReLU per
                position, even positions on ScalarE and odd on VectorE, each
                parity writing its OWN output tile so the two engines never
                falsely serialize on a shared destination."""
                for pp in range(nslab):
                    p = p0 + pp
                    pt = pss.tile([128, 1, 2, B], f32, tag="pts",
                                  name=f"pt{l}_{p}")
                    mm_pos(pt, 0, ws, woff0 + pp * WPP, xin, p)
                    dst = (ev if p % 2 == 0 else od)[:, p // 2, :, :]
                    if p % 2 == 0:
                        nc.scalar.activation(dst, pt[:, 0], Relu, scale=s16)
                    else:
                        nc.vector.tensor_scalar(
                            dst, pt[:, 0], s16, 0.0,
                            mybir.AluOpType.mult, mybir.AluOpType.max,
                        )

            # ---- layers 1-6: stream weight slabs (sync queue) through the
            # wp ring -- the ring-slot waits keep the DMA stream in
            # consumption order (the scheduler would hoist dep-free DMAs)
            X = X1
            for l in range(1, 7):
                n, slab = NPOS[l], SLAB[l]
                if n > 8:
                    Xout = sb.tile(
                        [128, n, 2, B], f16, tag=f"x{l + 1}", name=f"X{l + 1}"
                    )
                else:
                    ev, od = split_tiles(l, n, f"x{l + 1}")
                for s in range(n // slab):
                    ws = wp.tile([128, slab * WPP], f16, tag="ws", name=f"ws{l}_{s}")
                    # the very first slab goes via SWDGE (gpsimd) so its
                    # descriptor setup overlaps xw0's HWDGE setup at t=0
                    eng = nc.gpsimd if (l == 1 and s == 0) else nc.sync
                    eng.dma_start(
                        ws[:], w_d[l][:, s * slab * WPP : (s + 1) * slab * WPP]
                    )
                    if n > 8:
                        lc_wide(l, X, Xout, ws, s * slab, slab)
                    else:
                        lc_split(l, X, ws, 0, s * slab, slab, ev, od)
                X = Xout if n > 8 else (ev if od is None else (ev, od))
            # tail weights (layers 7-9, replicated) ride the same ring: the
            # slot wait orders them near the stream's end, overlapping the
            # layer 4-6 compute chain and done well before the collective
            # (two chunks so the ag_in hop can slot between them on the pipe)
            wtail = wp.tile([128, 7 * WPP], f16, tag="ws", name="wtail_sb")
            for c0, c1 in ((0, 3), (3, 5), (5, 7)):
                nc.sync.dma_start(
                    wtail[:, c0 * WPP : c1 * WPP], wtail_d[:, c0 * WPP : c1 * WPP]
                )

            # ---- AllGather the single layer-6 output position across cores
            ag_in = dp.tile([128, 2, B], f16, name="ag_in")
            nc.sync.dma_start(ag_in[:], X[:, 0, :, :])
            if sim:
                # timing stub (single-core twin): the collective is replaced
                # by a dram->dram copy carrying the same ag_in -> ag_out
                # dependency, matching the convention the baseline number
                # was measured under.
                ag_out = dp.tile([N_CORES * 128, 2, B], f16, name="ag_out")
                nc.sync.dma_start(ag_out[0:128, :, :], ag_in[:])
            else:
                ag_out = dp.tile(
                    [N_CORES * 128, 2, B], f16, addr_space="Shared", name="ag_out"
                )
                nc.gpsimd.collective_compute(
                    "AllGather",
                    mybir.AluOpType.bypass,
                    replica_groups=[list(range(N_CORES))],
                    ins=[ag_in.opt()],
                    outs=[ag_out.opt()],
                )
            pe_fill(FILL_CC, dep=X[:, 0, 0, :], rhs=X1[:, 0:2, :, :])
            X7 = sb.tile([128, 8, 2, B], f16, tag="x7g", name="X7")
            agv = ag_out.rearrange("(pos p) ch b -> p pos ch b", pos=N_CORES)
            # four chunks: layer 7's position p needs only gathered positions
            # 2p/2p+1, so each chunk unblocks one tail position early
            for c in range(4):
                nc.sync.dma_start(
                    X7[:, 2 * c : 2 * c + 2, :, :], agv[:, 2 * c : 2 * c + 2, :, :]
                )

            # ---- layers 7-9 (replicated tail), weights already resident
            X = X7
            off = 0
            for l in range(7, 10):
                n = NPOS[l]
                ev, od = split_tiles(l, n, f"x{l + 1}")
                lc_split(l, X, wtail, off * WPP, 0, n, ev, od)
                X = ev if od is None else (ev, od)
                off += n

            # ---- head: out[b, j] = sum_c X10[c, b] * beta[c, j] (the /256
            # is folded into beta host-side)
            # reuses the filler bank: the first head matmul's start=True
            # resets the region after the last filler wrote it
            ph = psf.tile([128, OUT], f32, tag="fill", name="ph")
            for ch in range(2):
                nc.tensor.matmul(
                    ph[:],
                    X[:, 0, ch, :],
                    beta_sb[:, ch * OUT : (ch + 1) * OUT],
                    start=(ch == 0),
                    stop=(ch == 1),
                )
            ob = yp.tile([128, OUT], f32, tag="ob", name="ob")
            nc.scalar.copy(ob[:], ph[:])
            nc.sync.dma_start(out_d[:], ob[:])

    nc.compile()
    return nc


def _get_nc(sim=False):
    if sim not in _NC:
        _NC[sim] = _build(sim)
    return _NC[sim]


def _prep(inputs):
    x = np.asarray(inputs["x"], dtype=np.float32)
    beta = np.asarray(inputs["beta"], dtype=np.float32)
    ws = [np.asarray(inputs[f"w{l}"], dtype=np.float32) for l in range(10)]

    # x (B,3,1024) -> (j=kk*3+c, d=512, b) fp16
    xk = np.ascontiguousarray(
        x.reshape(B, 3, 512, 2).transpose(3, 1, 2, 0).reshape(6, 512, B)
    ).astype(np.float16)
    # w0 (256,3,512,2) -> (j, d, o) fp16
    w0t = np.ascontiguousarray(
        ws[0].transpose(3, 1, 2, 0).reshape(6, 512, H)
    ).astype(np.float16)
    xk = xk.reshape(6, 8, 64, B)
    w0t = w0t.reshape(6, 8, 64, H)

    # wl (256,256,dl,2) -> [cc=128, (pos, oh, kk, ch, o=128)] fp16
    slabs = {}
    for l in range(1, 10):
        w = ws[l]
        dl = w.shape[2]
        wt = w.reshape(2, 128, 2, 128, dl, 2)  # (oh, oo, ch, cc, d, kk)
        wt = wt.transpose(3, 4, 0, 5, 2, 1)  # (cc, d, oh, kk, ch, oo)
        slabs[l] = np.ascontiguousarray(wt).reshape(128, dl * WPP).astype(np.float16)

    wtail = np.concatenate([slabs[7], slabs[8], slabs[9]], axis=1)
    beta2 = (
        np.concatenate([beta[:128], beta[128:]], axis=1) / 256.0
    ).astype(np.float16)

    in_maps = []
    for i in range(N_CORES):
        xw0 = np.concatenate(
            [xk[:, i].reshape(6, 64 * B), w0t[:, i].reshape(6, 64 * H)], axis=1
        )
        m = {
            "xw0": np.ascontiguousarray(xw0),
            "beta": beta2,
            "wtail": wtail,
        }
        for l in range(1, 7):
            n = NPOS[l]
            m[f"w{l}"] = np.ascontiguousarray(
                slabs[l][:, i * n * WPP : (i + 1) * n * WPP]
            )
        in_maps.append(m)
    return in_maps


def _run(in_maps, trace=False):
    from concourse import bass_utils

    return bass_utils.run_bass_kernel_spmd(
        _get_nc(), in_maps, core_ids=list(range(N_CORES)), trace=trace
    )


def kernel(**inputs):
    res = _run(_prep(inputs))
    return np.asarray(res.results[0]["out"], dtype=np.float32)
